# revision 18
# baseline (speedup 1.0000x reference)
"""MetaLSTMCell Trainium2 kernel: 8-core tensor-parallel over the hidden dim.

Sharding: each core owns a 128-column slice of H for all 4 gates.
Algebraic fold: the hypernetwork projections (zh/zx/zb) are folded into
effective matrices M_* = d*_w[g,hs,:] @ z*_w_g  (computed on device), so the
per-core GEMMs are:
    D_* = src_meta @ M_*^T (+bias)   [B, 4*128]
    W_H = h @ w_h_slice^T, W_X = x @ w_x_slice^T
    y = D_H*W_H + D_X*W_X + D_B
LayerNorm needs full-H moments -> tiny AllReduce of per-core (sum, sumsq).
Gate blocks are host-permuted to [i, f, o, g] so sigmoid runs as one
[128,384] activation and tanh as one [128,128].
"""

import sys

sys.path.insert(0, "/opt/trn_rl_repo")

from contextlib import ExitStack

import numpy as np
import concourse.bass as bass
import concourse.mybir as mybir
import concourse.tile as tile
from concourse.bass_utils import run_bass_kernel_spmd

B, IN, H, Z, G = 2048, 1024, 1024, 256, 4
NCORES, HS = 8, 128
N = G * HS            # 512
BT = 128              # batch tile
NBT = B // BT         # 16
NCH = 4               # allreduce chunks
TPC = NBT // NCH      # tiles per chunk
PERM = (0, 1, 3, 2)   # gate order [i, f, o, g]: sigmoid on cols 0:384

dt = mybir.dt
AF = mybir.ActivationFunctionType
ALU = mybir.AluOpType
F32, BF16 = dt.float32, dt.bfloat16


def fixup_multi_waits(nc):
    """This toolchain's walrus accepts at most ONE sync wait per instruction;
    Tile emits several. Hoist extras onto same-engine NOPs placed before."""
    for f in nc.m.functions:
        for blk in f.blocks:
            out = []
            changed = False
            for inst in blk.instructions:
                si = getattr(inst, "sync_info", None)
                waits = list(si.on_wait) if si is not None and si.on_wait else []
                if len(waits) > 1:
                    changed = True
                    for k, w in enumerate(waits[:-1]):
                        nop = mybir.InstNoOp(
                            name=f"{inst.name}-waitsplit{k}", ins=[], outs=[]
                        )
                        nop.engine = inst.engine
                        nop.sync_info = mybir.SyncInfo(on_wait=[w], on_update=[])
                        out.append(nop)
                    si.on_wait = [waits[-1]]
                out.append(inst)
            if changed:
                blk.instructions = out


def build():
    nc = bass.Bass(trn_type="TRN2", num_devices=NCORES)
    P = 128

    def din(name, shape):
        return nc.dram_tensor(name, shape, F32, kind="ExternalInput")

    xT = din("xT", [IN, B])
    hT = din("hT", [IN, B])
    mT = din("mT", [Z, B])
    c_s = din("c_s", [B, HS])
    whT = din("whT", [IN, N])
    wxT = din("wxT", [IN, N])
    zhw = din("zhw", [G * Z, Z])
    zxw = din("zxw", [G * Z, Z])
    zbw = din("zbw", [G * Z, Z])
    dhwT = din("dhwT", [G * Z, HS])
    dxwT = din("dxwT", [G * Z, HS])
    dbwT = din("dbwT", [G * Z, HS])
    bdh = din("bdh", [1, N])
    bdx = din("bdx", [1, N])
    dbb = din("dbb", [1, N])
    lnw = din("lnw", [1, N])
    lnb = din("lnb", [1, N])
    hn = nc.dram_tensor("hn", [B, HS], F32, kind="ExternalOutput")
    cn = nc.dram_tensor("cn", [B, HS], F32, kind="ExternalOutput")

    with tile.TileContext(nc) as tc:
        with tc.tile_pool(name="wres", bufs=1) as wres, \
             tc.tile_pool(name="dram", bufs=1, space="DRAM") as dram, \
             tc.tile_pool(name="stream", bufs=3) as sp, \
             tc.tile_pool(name="sact", bufs=3) as sa, \
             tc.tile_pool(name="ypool", bufs=NBT) as yp, \
             tc.tile_pool(name="cpool", bufs=NBT) as cp, \
             tc.tile_pool(name="phb", bufs=3) as pb, \
             tc.tile_pool(name="psd", bufs=3, space="PSUM") as psd, \
             tc.tile_pool(name="psw", bufs=5, space="PSUM") as psw:

            whT_b = wres.tile([P, IN // P, N], BF16)
            wxT_b = wres.tile([P, IN // P, N], BF16)
            MhT = wres.tile([P, 2, N], BF16)
            MxT = wres.tile([P, 2, N], BF16)
            MbT = wres.tile([P, 2, N], BF16)
            rep_lnw = wres.tile([P, N], F32)
            rep_lnb = wres.tile([P, N], F32)
            eps_t = wres.tile([P, 1], F32)
            nc.vector.memset(eps_t[:], 1e-5)
            # ones-in-row-0 stationary operand: matmul with it broadcast-adds
            # the rhs row 0 (bias rows) into an accumulating psum
            e0 = wres.tile([P, P], BF16)
            nc.vector.memset(e0[:], 0.0)
            nc.vector.memset(e0[:1, :], 1.0)
            bias3h = wres.tile([P, N], BF16)
            bias3x = wres.tile([P, N], BF16)
            bias3b = wres.tile([P, N], BF16)
            for t_ in (bias3h, bias3x, bias3b):
                nc.vector.memset(t_[:], 0.0)

            mom_in = dram.tile([B, 8], F32)
            mom_out = dram.tile([B, 8], F32)

            # main-GEMM weights: load + cast per K-chunk so PE can start early
            for (w_d, w_b, tg) in ((whT, whT_b, "whst"), (wxT, wxT_b, "wxst")):
                for kc in range(IN // P):
                    wst = sa.tile([P, N], F32, tag=tg)
                    nc.sync.dma_start(
                        wst[:],
                        w_d.ap().rearrange("(k p) n -> p k n", p=P)[:, kc])
                    nc.scalar.copy(w_b[:, kc], wst[:])

            ytiles = []
            ctiles = []

            def phase_a1(bt):
                """activation loads + casts + the two big GEMMs"""
                bs = slice(bt * BT, (bt + 1) * BT)
                xs = sa.tile([P, IN // P, BT], F32, tag="xs")
                nc.sync.dma_start(
                    xs[:], xT.ap().rearrange("(k p) b -> p k b", p=P)[:, :, bs])
                xb = sa.tile([P, IN // P, BT], BF16, tag="xb")
                nc.scalar.copy(xb[:], xs[:])
                hs_ = sa.tile([P, IN // P, BT], F32, tag="hs")
                nc.sync.dma_start(
                    hs_[:], hT.ap().rearrange("(k p) b -> p k b", p=P)[:, :, bs])
                hb = sa.tile([P, IN // P, BT], BF16, tag="hb")
                nc.scalar.copy(hb[:], hs_[:])
                ms = sa.tile([P, Z // P, BT], F32, tag="ms")
                nc.sync.dma_start(
                    ms[:], mT.ap().rearrange("(k p) b -> p k b", p=P)[:, :, bs])
                mb = sa.tile([P, Z // P, BT], BF16, tag="mb")
                nc.gpsimd.tensor_copy(mb[:], ms[:])

                c_t = cp.tile([P, HS], F32, tag="ct")
                nc.sync.dma_start(c_t[:], c_s[bs, :])
                ctiles.append(c_t)

                WH = psw.tile([P, N], F32, tag="psw")
                for kc in range(IN // P):
                    nc.tensor.matmul(WH[:], hb[:, kc], whT_b[:, kc],
                                     start=(kc == 0), stop=(kc == IN // P - 1))
                WX = psw.tile([P, N], F32, tag="psw")
                for kc in range(IN // P):
                    nc.tensor.matmul(WX[:], xb[:, kc], wxT_b[:, kc],
                                     start=(kc == 0), stop=(kc == IN // P - 1))
                return mb, WH, WX

            def phase_a2(bt, mb, WH, WX):
                """small GEMMs (bias folded in as a 3rd K-chunk) + y assembly"""
                DH = psd.tile([P, N], F32, tag="psd")
                DX = psd.tile([P, N], F32, tag="psd")
                DB = psd.tile([P, N], F32, tag="psd")
                for (D, MT, b3) in ((DH, MhT, bias3h), (DX, MxT, bias3x),
                                    (DB, MbT, bias3b)):
                    for kc in range(Z // P):
                        nc.tensor.matmul(D[:], mb[:, kc], MT[:, kc],
                                         start=(kc == 0), stop=False)
                    nc.tensor.matmul(D[:], e0[:], b3[:], start=False, stop=True)

                # PSUM -> SBUF moves on ACT so DVE only does the math
                dh_s = sp.tile([P, N], F32, tag="dh_s")
                nc.scalar.copy(dh_s[:], DH[:])
                dx_s = sp.tile([P, N], F32, tag="dx_s")
                nc.scalar.copy(dx_s[:], DX[:])
                y1 = sp.tile([P, N], F32, tag="y1")
                nc.vector.tensor_mul(y1[:], WH[:], dh_s[:])
                y2 = sp.tile([P, N], F32, tag="y2")
                nc.vector.tensor_mul(y2[:], WX[:], dx_s[:])
                y3 = sp.tile([P, N], F32, tag="y3")
                nc.vector.tensor_add(y3[:], y1[:], y2[:])

                y = yp.tile([P, N], BF16, tag="y")
                mom = sp.tile([P, 8], F32, tag="mom")
                ysq = sp.tile([P, N], BF16, tag="ysq")
                for g in range(G):
                    gs = slice(g * HS, (g + 1) * HS)
                    # y = y3 + DB, accumulating per-gate sums on the fly
                    nc.vector.scalar_tensor_tensor(
                        y[:, gs], y3[:, gs], 1.0, DB[:, gs],
                        ALU.mult, ALU.add, accum_out=mom[:, g:g + 1])
                nc.gpsimd.tensor_mul(ysq[:], y[:], y[:])
                for g in range(G):
                    nc.vector.reduce_sum(mom[:, 4 + g:5 + g],
                                         ysq[:, g * HS:(g + 1) * HS],
                                         axis=mybir.AxisListType.X)
                ytiles.append(y)
                nc.sync.dma_start(mom_in[bt * BT:(bt + 1) * BT, :], mom[:])

            def phase_b(bt):
                bs = slice(bt * BT, (bt + 1) * BT)
                y = ytiles[bt]
                c_t = ctiles[bt]
                gmom = pb.tile([P, 8], F32, tag="gmom")
                nc.sync.dma_start(gmom[:], mom_out[bs, :])
                scl = pb.tile([P, 8], F32, tag="scl")
                nc.vector.tensor_scalar_mul(scl[:], gmom[:], 1.0 / H)
                mu = scl[:, 0:4]
                var = pb.tile([P, 4], F32, tag="var")
                nc.vector.tensor_mul(var[:], mu, mu)
                nc.vector.tensor_sub(var[:], scl[:, 4:8], var[:])
                sq = pb.tile([P, 4], F32, tag="sq")
                nc.scalar.activation(sq[:], var[:], AF.Sqrt, bias=eps_t[:])
                rs = pb.tile([P, 4], F32, tag="rs")
                nc.vector.reciprocal(rs[:], sq[:])
                nmrs = pb.tile([P, 4], F32, tag="nmrs")
                nc.vector.scalar_tensor_tensor(
                    nmrs[:], mu, -1.0, rs[:], ALU.mult, ALU.mult)

                vv = pb.tile([P, N], F32, tag="vv")
                for g in range(G):
                    gs = slice(g * HS, (g + 1) * HS)
                    nc.vector.tensor_scalar(
                        vv[:, gs], y[:, gs], rs[:, g:g + 1], nmrs[:, g:g + 1],
                        op0=ALU.mult, op1=ALU.add)
                nc.gpsimd.tensor_mul(vv[:], vv[:], rep_lnw[:])
                nc.gpsimd.tensor_add(vv[:], vv[:], rep_lnb[:])
                gt = pb.tile([P, N], F32, tag="gt")
                nc.scalar.activation(gt[:, 0:3 * HS], vv[:, 0:3 * HS], AF.Sigmoid)
                nc.scalar.activation(gt[:, 3 * HS:N], vv[:, 3 * HS:N], AF.Tanh)

                # gate order [i, f, o, g]
                sfc = pb.tile([P, HS], F32, tag="sfc")
                nc.vector.tensor_mul(sfc[:], gt[:, HS:2 * HS], c_t[:])
                sit = pb.tile([P, HS], F32, tag="sit")
                nc.vector.tensor_mul(sit[:], gt[:, 0:HS], gt[:, 3 * HS:N])
                cn_t = pb.tile([P, HS], F32, tag="cn_t")
                nc.vector.tensor_add(cn_t[:], sfc[:], sit[:])
                tc_t = pb.tile([P, HS], F32, tag="tc_t")
                nc.scalar.activation(tc_t[:], cn_t[:], AF.Tanh)
                hn_t = pb.tile([P, HS], F32, tag="hn_t")
                nc.gpsimd.tensor_mul(hn_t[:], gt[:, 2 * HS:3 * HS], tc_t[:])
                nc.sync.dma_start(cn[bs, :], cn_t[:])
                nc.sync.dma_start(hn[bs, :], hn_t[:])

            # ---- tile 0 big GEMMs first, M-precompute overlaps them on PE
            a1_0 = phase_a1(0)

            with ExitStack() as pre_ctx:
                pre = pre_ctx.enter_context(tc.tile_pool(name="pre", bufs=1))
                for (zw_d, dw_d, MT) in ((zhw, dhwT, MhT), (zxw, dxwT, MxT),
                                         (zbw, dbwT, MbT)):
                    zst = pre.tile([P, 2 * G, Z], F32, tag="zstage")
                    nc.sync.dma_start(
                        zst[:], zw_d.ap().rearrange("(c p) z -> p c z", p=P))
                    zbf = pre.tile([P, 2 * G, Z], BF16, tag="zbf")
                    nc.scalar.copy(zbf[:], zst[:])
                    dst = pre.tile([P, 2 * G, HS], F32, tag="dstage")
                    nc.sync.dma_start(
                        dst[:], dw_d.ap().rearrange("(c p) n -> p c n", p=P))
                    dbf = pre.tile([P, 2 * G, HS], BF16, tag="dbf")
                    nc.scalar.copy(dbf[:], dst[:])
                    for g in range(G):
                        for zmc in range(2):
                            ps = psd.tile([P, HS], F32, tag="psd")
                            for zc in range(2):
                                nc.tensor.matmul(
                                    ps[:],
                                    zbf[:, g * 2 + zc, zmc * P:(zmc + 1) * P],
                                    dbf[:, g * 2 + zc],
                                    start=(zc == 0), stop=(zc == 1),
                                )
                            nc.vector.tensor_copy(
                                MT[:, zmc, g * HS:(g + 1) * HS], ps[:])

                # bias rows -> row 0 of the bias3 stationary-rhs tiles (bf16)
                for (row_d, b3) in ((bdh, bias3h), (bdx, bias3x), (dbb, bias3b)):
                    rowt = pre.tile([1, N], F32, tag="rowt")
                    nc.sync.dma_start(rowt[:], row_d.ap())
                    nc.vector.tensor_copy(b3[:1, :], rowt[:])

                # broadcast ln rows to [128,N] via fp32 matmul with ones
                ones = pre.tile([1, P], F32, tag="ones")
                nc.vector.memset(ones[:], 1.0)
                for (row_d, rep) in ((lnw, rep_lnw), (lnb, rep_lnb)):
                    rowt = pre.tile([1, N], F32, tag="rowt")
                    nc.sync.dma_start(rowt[:], row_d.ap())
                    bp = psd.tile([P, N], F32, tag="psd")
                    nc.tensor.matmul(bp[:], ones[:], rowt[:], start=True, stop=True)
                    nc.vector.tensor_copy(rep[:], bp[:])

            phase_a2(0, *a1_0)
            for bt in range(1, TPC):
                phase_a2(bt, *phase_a1(bt))

            def do_ar(ch):
                rows = slice(ch * TPC * BT, (ch + 1) * TPC * BT)
                nc.gpsimd.collective_compute(
                    "AllReduce", ALU.add,
                    replica_groups=[list(range(NCORES))],
                    ins=[mom_in[rows, :]],
                    outs=[mom_out[rows, :]],
                )

            do_ar(0)
            for ch in range(1, NCH):
                for bt in range(ch * TPC, (ch + 1) * TPC):
                    phase_a2(bt, *phase_a1(bt))
                do_ar(ch)
                for bt in range((ch - 1) * TPC, ch * TPC):
                    phase_b(bt)
            for bt in range((NCH - 1) * TPC, NBT):
                phase_b(bt)

    fixup_multi_waits(nc)
    return nc


_nc = None


def _get_nc():
    global _nc
    if _nc is None:
        _nc = build()
    return _nc


def make_in_maps(src_x, h, c, src_meta, zh_w, zh_b, zx_w, zx_b, zb_w,
                 dh_w, dx_w, db_w, db_b, w_h, w_x, ln_w, ln_b):
    f32 = np.float32
    asc = np.ascontiguousarray
    perm = list(PERM)
    # permute gate blocks to [i, f, o, g]
    w_h = w_h[perm]
    w_x = w_x[perm]
    dh_w = dh_w[perm]
    dx_w = dx_w[perm]
    db_w = db_w[perm]
    db_b = db_b[perm]
    ln_w = ln_w[perm]
    ln_b = ln_b[perm]
    zh_w = zh_w.reshape(G, Z, Z)[perm].reshape(G * Z, Z)
    zx_w = zx_w.reshape(G, Z, Z)[perm].reshape(G * Z, Z)
    zb_w = zb_w.reshape(G, Z, Z)[perm].reshape(G * Z, Z)
    zh_b2 = zh_b.reshape(G, Z)[perm]
    zx_b2 = zx_b.reshape(G, Z)[perm]

    xT = asc(src_x.T.astype(f32, copy=False))
    hT = asc(h.T.astype(f32, copy=False))
    mT = asc(src_meta.T.astype(f32, copy=False))
    in_maps = []
    for ci in range(NCORES):
        hs = slice(ci * HS, (ci + 1) * HS)
        whT_c = asc(w_h[:, hs, :].transpose(2, 0, 1).reshape(IN, N))
        wxT_c = asc(w_x[:, hs, :].transpose(2, 0, 1).reshape(IN, N))
        dhwT_c = asc(dh_w[:, hs, :].transpose(0, 2, 1).reshape(G * Z, HS))
        dxwT_c = asc(dx_w[:, hs, :].transpose(0, 2, 1).reshape(G * Z, HS))
        dbwT_c = asc(db_w[:, hs, :].transpose(0, 2, 1).reshape(G * Z, HS))
        bdh_c = np.einsum("gz,ghz->gh", zh_b2, dh_w[:, hs, :]).astype(f32)
        bdx_c = np.einsum("gz,ghz->gh", zx_b2, dx_w[:, hs, :]).astype(f32)
        in_maps.append({
            "xT": xT, "hT": hT, "mT": mT,
            "c_s": asc(c[:, hs]),
            "whT": whT_c, "wxT": wxT_c,
            "zhw": asc(zh_w), "zxw": asc(zx_w), "zbw": asc(zb_w),
            "dhwT": dhwT_c, "dxwT": dxwT_c, "dbwT": dbwT_c,
            "bdh": asc(bdh_c.reshape(1, N)),
            "bdx": asc(bdx_c.reshape(1, N)),
            "dbb": asc(db_b[:, hs].reshape(1, N)),
            "lnw": asc(ln_w[:, hs].reshape(1, N)),
            "lnb": asc(ln_b[:, hs].reshape(1, N)),
        })
    return in_maps


def run(inputs, trace=False):
    nc = _get_nc()
    in_maps = make_in_maps(**inputs)
    res = run_bass_kernel_spmd(nc, in_maps, core_ids=list(range(NCORES)),
                               trace=trace)
    h_next = np.concatenate([res.results[i]["hn"] for i in range(NCORES)], axis=1)
    c_next = np.concatenate([res.results[i]["cn"] for i in range(NCORES)], axis=1)
    return (h_next, c_next), res


def kernel(**inputs):
    (h_next, c_next), _ = run(inputs, trace=False)
    return (h_next, c_next)
